# revision 10
# baseline (speedup 1.0000x reference)
"""MoE (dense all-expert forward, top-2 gating) on 8 TRN2 NeuronCores.

Strategy: data-parallel over the token axis. Each core owns N/8 = 1024
tokens and runs all 8 experts densely on its shard (matches the
reference, which computes every expert for every token and combines
with the sparse top-2 gates). Weights are replicated to every core.

Per-core device program (all compute on device):
  - gating: logits = x @ wg_w + wg_b   -> top-2 mask -> softmax -> gates
    (logits in true fp32: walrus picks the matmul path from the
    *allocation* dtype, and f32r's ~4e-4 error flips near-tied top-2
    selections)
  - per expert e: h = relu(xT.T @ W1[e] + b1[e]) (layout [H, T]),
    y_e = h.T @ W2[e] (layout [T, D]), y += gates[:, e] * (y_e + b2[e])
  - outputs: y shard [1024, 1024], gates shard [1024, 8]

FFN matmuls run in float32r (fp32 data streamed at bf16 rate for free
dim >= 256). The scalar load-balancing loss is computed on the host
from the gates output (a trivial reduction over a [8192, 8] array).
"""

import os
import sys

sys.path.insert(0, "/opt/trn_rl_repo")

import numpy as np

import concourse.bass as bass  # noqa: F401
import concourse.mybir as mybir
import concourse.tile as tile
from concourse import bacc
from concourse.bass_utils import run_bass_kernel_spmd

N, D, H, E = 8192, 1024, 1024, 8
NCORES = 8
T = N // NCORES  # 1024 tokens per core
P = 128
TT = T // P  # token tiles per core
DT = D // P
HT = H // P
FH = 512  # psum-bank free chunk (512 fp32)
LMBDA = 0.01

FR = mybir.dt.float32r
F32 = mybir.dt.float32

LAST = {}

_prog = {}


def _build_program(repeat=1):
    nc = bacc.Bacc(
        "TRN2", target_bir_lowering=False, debug=False, num_devices=NCORES
    )

    xT_d = nc.dram_tensor("xT", [D, T], FR, kind="ExternalInput")
    wg_d = nc.dram_tensor("wg", [D, E], F32, kind="ExternalInput")
    wgb_d = nc.dram_tensor("wgb", [P, E], F32, kind="ExternalInput")
    w1_d = nc.dram_tensor("w1", [E, D, H], FR, kind="ExternalInput")
    b1_d = nc.dram_tensor("b1", [E, H], F32, kind="ExternalInput")
    w2_d = nc.dram_tensor("w2", [E, H, D], FR, kind="ExternalInput")
    b2b_d = nc.dram_tensor("b2b", [E, P, D], F32, kind="ExternalInput")
    y_d = nc.dram_tensor("y", [T, D], F32, kind="ExternalOutput")
    g_d = nc.dram_tensor("gates", [T, E], F32, kind="ExternalOutput")

    with tile.TileContext(nc) as tc:
        with (
            tc.tile_pool(name="xt", bufs=1) as xt_pool,
            tc.tile_pool(name="wpool", bufs=2) as wpool,
            tc.tile_pool(name="hpool", bufs=1) as hpool,
            tc.tile_pool(name="ypool", bufs=1) as ypool,
            tc.tile_pool(name="b2pool", bufs=2) as b2pool,
            tc.tile_pool(name="small", bufs=1) as small,
            tc.tile_pool(name="gtmp", bufs=2) as gtmp,
            tc.tile_pool(name="pg", bufs=2, space="PSUM") as pg,
            tc.tile_pool(name="ph", bufs=3, space="PSUM") as ph,
            tc.tile_pool(name="py", bufs=3, space="PSUM") as py,
        ):

            def body_once():
                # --- resident tiles ---
                xt = xt_pool.tile([P, DT, T], FR)  # x shard transposed [D, T]
                for dt in range(DT):
                    nc.sync.dma_start(
                        xt[:, dt, :], xT_d[dt * P : (dt + 1) * P, :]
                    )

                wg_sb = small.tile([P, DT, E], F32)
                nc.sync.dma_start(
                    wg_sb[:], wg_d.rearrange("(dt p) e -> p dt e", p=P)
                )
                wgb_sb = small.tile([P, E], F32)
                nc.sync.dma_start(wgb_sb[:], wgb_d[:])

                gates_sb = small.tile([P, TT, E], F32)
                y_acc = ypool.tile([P, TT, D], F32)

                # --- gating: logits -> top-2 softmax gates ---
                for ti in range(TT):
                    # true-fp32 copy of this token tile for the logits
                    xt32 = gtmp.tile([P, DT, P], F32, tag="xt32")
                    for dt in range(DT):
                        nc.sync.dma_start(
                            xt32[:, dt, :],
                            xT_d[
                                dt * P : (dt + 1) * P, ti * P : (ti + 1) * P
                            ].bitcast(F32),
                        )
                    lg = pg.tile([P, E], F32)
                    for dt in range(DT):
                        nc.tensor.matmul(
                            lg[:],
                            xt32[:, dt, :],
                            wg_sb[:, dt, :],
                            start=(dt == 0),
                            stop=(dt == DT - 1),
                        )
                    logit = gtmp.tile([P, E], F32)
                    nc.vector.tensor_tensor(
                        logit[:], lg[:], wgb_sb[:], mybir.AluOpType.add
                    )
                    m1 = gtmp.tile([P, 1], F32)
                    nc.vector.reduce_max(
                        m1[:], logit[:], axis=mybir.AxisListType.X
                    )
                    tsub = gtmp.tile([P, E], F32)
                    nc.vector.tensor_scalar_sub(tsub[:], logit[:], m1[:])
                    eq = gtmp.tile([P, E], F32)
                    nc.vector.tensor_scalar(
                        eq[:], tsub[:], 0.0, None, op0=mybir.AluOpType.is_equal
                    )
                    msk = gtmp.tile([P, E], F32)
                    nc.vector.scalar_tensor_tensor(
                        msk[:],
                        eq[:],
                        -1e30,
                        tsub[:],
                        op0=mybir.AluOpType.mult,
                        op1=mybir.AluOpType.add,
                    )
                    m2 = gtmp.tile([P, 1], F32)
                    nc.vector.reduce_max(
                        m2[:], msk[:], axis=mybir.AxisListType.X
                    )
                    keep = gtmp.tile([P, E], F32)
                    nc.vector.tensor_scalar(
                        keep[:], tsub[:], m2[:], None, op0=mybir.AluOpType.is_ge
                    )
                    ex = gtmp.tile([P, E], F32)
                    nc.scalar.activation(
                        ex[:], tsub[:], mybir.ActivationFunctionType.Exp
                    )
                    ek = gtmp.tile([P, E], F32)
                    nc.vector.tensor_tensor(
                        ek[:], ex[:], keep[:], mybir.AluOpType.mult
                    )
                    s = gtmp.tile([P, 1], F32)
                    nc.vector.reduce_sum(
                        s[:], ek[:], axis=mybir.AxisListType.X
                    )
                    r = gtmp.tile([P, 1], F32)
                    nc.vector.reciprocal(r[:], s[:])
                    nc.vector.tensor_scalar_mul(
                        gates_sb[:, ti, :], ek[:], r[:]
                    )
                    nc.sync.dma_start(
                        g_d[ti * P : (ti + 1) * P, :], gates_sb[:, ti, :]
                    )

                # --- experts ---
                for e in range(E):
                    w1t = wpool.tile([P, DT, H], FR, tag="w")
                    for dt in range(DT):
                        nc.sync.dma_start(
                            w1t[:, dt, :], w1_d[e, dt * P : (dt + 1) * P, :]
                        )
                    b1t = gtmp.tile([P, HT], F32, tag="b1")
                    nc.sync.dma_start(
                        b1t[:], b1_d[e].rearrange("(ht p) -> p ht", p=P)
                    )

                    # phase A: h = relu(W1[e].T @ x.T) in [H, T] layout
                    h_sb = hpool.tile([P, HT, T], FR)
                    for ht in range(HT):
                        for th in range(T // FH):
                            hp = ph.tile([P, FH], F32)
                            for dt in range(DT):
                                nc.tensor.matmul(
                                    hp[:],
                                    w1t[:, dt, ht * P : (ht + 1) * P],
                                    xt[:, dt, th * FH : (th + 1) * FH],
                                    start=(dt == 0),
                                    stop=(dt == DT - 1),
                                )
                            nc.scalar.activation(
                                h_sb[:, ht, th * FH : (th + 1) * FH],
                                hp[:],
                                mybir.ActivationFunctionType.Relu,
                                bias=b1t[:, ht : ht + 1],
                                scale=1.0,
                            )

                    # phase B: y_e = h.T @ W2[e], gate-scale, accumulate
                    w2t = wpool.tile([P, HT, D], FR, tag="w")
                    for ht in range(HT):
                        nc.sync.dma_start(
                            w2t[:, ht, :], w2_d[e, ht * P : (ht + 1) * P, :]
                        )
                    b2t = b2pool.tile([P, D], F32)
                    nc.sync.dma_start(b2t[:], b2b_d[e])

                    for dh in range(D // FH):
                        for ti in range(TT):
                            yp = py.tile([P, FH], F32)
                            for ht in range(HT):
                                nc.tensor.matmul(
                                    yp[:],
                                    h_sb[:, ht, ti * P : (ti + 1) * P],
                                    w2t[:, ht, dh * FH : (dh + 1) * FH],
                                    start=(ht == 0),
                                    stop=(ht == HT - 1),
                                )
                            g_col = gates_sb[:, ti, e : e + 1]
                            ysl = y_acc[:, ti, dh * FH : (dh + 1) * FH]
                            b2sl = b2t[:, dh * FH : (dh + 1) * FH]
                            if e == 0:
                                nc.vector.tensor_scalar_mul(
                                    ysl, yp[:], g_col
                                )
                            else:
                                nc.vector.scalar_tensor_tensor(
                                    ysl,
                                    yp[:],
                                    g_col,
                                    ysl,
                                    op0=mybir.AluOpType.mult,
                                    op1=mybir.AluOpType.add,
                                )
                            nc.vector.scalar_tensor_tensor(
                                ysl,
                                b2sl,
                                g_col,
                                ysl,
                                op0=mybir.AluOpType.mult,
                                op1=mybir.AluOpType.add,
                            )
                            if e == E - 1:
                                nc.sync.dma_start(
                                    y_d[
                                        ti * P : (ti + 1) * P,
                                        dh * FH : (dh + 1) * FH,
                                    ],
                                    ysl,
                                )

            for _rep in range(repeat):
                body_once()

    nc.compile()
    return nc


def _get_program(repeat=1):
    if repeat not in _prog:
        _prog[repeat] = _build_program(repeat)
    return _prog[repeat]


def _make_in_maps(x, wg_w, wgb, W1, b1, W2, b2b):
    in_maps = []
    for c in range(NCORES):
        shard = x[c * T : (c + 1) * T]
        in_maps.append(
            {
                "xT": np.ascontiguousarray(shard.T),
                "wg": wg_w,
                "wgb": wgb,
                "w1": W1,
                "b1": b1,
                "w2": W2,
                "b2b": b2b,
            }
        )
    return in_maps


def _prep(x, wg_w, wg_b, W1, b1, W2, b2):
    x = np.ascontiguousarray(np.asarray(x, dtype=np.float32))
    wg_w = np.ascontiguousarray(np.asarray(wg_w, dtype=np.float32))
    wg_b = np.asarray(wg_b, dtype=np.float32)
    W1 = np.ascontiguousarray(np.asarray(W1, dtype=np.float32))
    b1 = np.ascontiguousarray(np.asarray(b1, dtype=np.float32))
    W2 = np.ascontiguousarray(np.asarray(W2, dtype=np.float32))
    b2 = np.asarray(b2, dtype=np.float32)
    wgb = np.ascontiguousarray(np.broadcast_to(wg_b[None, :], (P, E)))
    b2b = np.ascontiguousarray(np.broadcast_to(b2[:, None, :], (E, P, D)))
    return _make_in_maps(x, wg_w, wgb, W1, b1, W2, b2b)


def bench(x, wg_w, wg_b, W1, b1, W2, b2, iters=50, repeat=1):
    """Time the NEFF with device-resident inputs (no per-call transfers).

    Axon dispatch overhead is ~1 ms/call, far above the kernel itself, so
    use two program variants (repeat=1 vs repeat=R) and take the marginal
    time per extra repeat to recover the true on-device execution time.

    Returns (per_iter_ns, outs) for this repeat count.
    """
    import time as _time

    import jax
    from jax.experimental.shard_map import shard_map
    from jax.sharding import Mesh, NamedSharding, PartitionSpec

    from concourse import bass2jax

    in_maps = _prep(x, wg_w, wg_b, W1, b1, W2, b2)

    nc = _get_program(repeat)
    bass2jax.install_neuronx_cc_hook()

    partition_name = (
        nc.partition_id_tensor.name if nc.partition_id_tensor else None
    )
    in_names, out_names, out_avals, zero_outs = [], [], [], []
    for alloc in nc.m.functions[0].allocations:
        if not isinstance(alloc, mybir.MemoryLocationSet):
            continue
        name = alloc.memorylocations[0].name
        if alloc.kind == "ExternalInput":
            if name != partition_name:
                in_names.append(name)
        elif alloc.kind == "ExternalOutput":
            shape = tuple(alloc.tensor_shape)
            dtype = mybir.dt.np(alloc.dtype)
            out_names.append(name)
            out_avals.append(jax.core.ShapedArray(shape, dtype))
            zero_outs.append(np.zeros(shape, dtype))
    n_params = len(in_names)
    in_names = in_names + out_names
    if partition_name is not None:
        in_names.append(partition_name)

    def _body(*args):
        operands = list(args)
        if partition_name is not None:
            operands.append(bass2jax.partition_id_tensor())
        outs = bass2jax._bass_exec_p.bind(
            *operands,
            out_avals=tuple(out_avals),
            in_names=tuple(in_names),
            out_names=tuple(out_names),
            lowering_input_output_aliases=(),
            sim_require_finite=True,
            sim_require_nnan=True,
            nc=nc,
        )
        return tuple(outs)

    devices = jax.devices()[:NCORES]
    mesh = Mesh(np.asarray(devices), ("core",))
    spec = PartitionSpec("core")
    jitted = jax.jit(
        shard_map(
            _body,
            mesh=mesh,
            in_specs=(spec,) * (n_params + len(out_names)),
            out_specs=(spec,) * len(out_names),
            check_rep=False,
        ),
        keep_unused=True,
    )
    sharding = NamedSharding(mesh, spec)
    concat_in = [
        jax.device_put(
            np.concatenate(
                [np.asarray(in_maps[c][nm]) for c in range(NCORES)], axis=0
            ),
            sharding,
        )
        for nm in in_names[:n_params]
    ]
    concat_zero = [
        jax.device_put(
            np.zeros((NCORES * z.shape[0], *z.shape[1:]), z.dtype), sharding
        )
        for z in zero_outs
    ]
    args = [*concat_in, *concat_zero]

    out = jitted(*args)
    jax.block_until_ready(out)  # compile + warm

    t0 = _time.perf_counter()
    for _ in range(iters):
        out = jitted(*args)
    jax.block_until_ready(out)
    per_iter_ns = (_time.perf_counter() - t0) * 1e9 / iters

    outs = {
        nm: np.asarray(out[i]).reshape(NCORES, *out_avals[i].shape)
        for i, nm in enumerate(out_names)
    }
    return per_iter_ns, outs


def kernel(x, wg_w, wg_b, W1, b1, W2, b2):
    in_maps = _prep(x, wg_w, wg_b, W1, b1, W2, b2)
    nc = _get_program(1)

    res = run_bass_kernel_spmd(nc, in_maps, core_ids=list(range(NCORES)))

    y = np.concatenate([res.results[c]["y"] for c in range(NCORES)], axis=0)
    gates = np.concatenate(
        [res.results[c]["gates"] for c in range(NCORES)], axis=0
    )

    importance = gates.mean(axis=0)
    loss = np.float32(LMBDA) * np.std(
        importance, ddof=1, dtype=np.float32
    ) / np.mean(importance, dtype=np.float32)

    return y, np.float32(loss), gates
